# revision 25
# baseline (speedup 1.0000x reference)
"""Causal self-attention (B=4, T=2048, C=1024, H=16, D=64) on 8 TRN2 NeuronCores.

Sharding: core c = 2*b + g handles batch b (0..3) and head-group g (0..1),
i.e. 8 heads per core (4 head-pairs). Column-parallel QKV, row-parallel
c_proj; host sums the two partial outputs per batch.

All matmul operands are bf16 (host pre-casts x and weights; DMA loads them
straight into SBUF with no staging copies), accumulation is fp32 in PSUM.
A/C projection phases share one 2-buffer PSUM pool so consecutive
accumulation groups ping-pong banks instead of serializing.

Per-core pipeline (chunk-pipelined A->B->C, interleaved at sub-tile
granularity so the PE/ACT streams stay dense):
  A(n): QKV projection for 512-token chunk n. qT/kT as [pair dims, tokens],
        v as [tokens, head, dim(+ones col)].
  B(n): flash attention, scores transposed attT[k, q]; exp on ACT
        (PSUM->SBUF, bf16 out), AV with ones-augmented V accumulating yT +
        denominator; deferred normalization via DVE reciprocal +
        GpSimd partition_broadcast + DVE multiply.
  C(n): row-parallel c_proj partial output for chunk n.

Biases: b_attn q/k parts added on-device (per-partition scalar add);
v-bias and b_proj folded into a host-side output correction.
"""

import numpy as np
import ml_dtypes

import concourse.bass as bass
from concourse import bacc, tile, mybir, bass_utils

P = 128
T = 2048
C = 1024
NH = 16          # total heads
D = 64
NCORES = 8
NCH = 4          # 512-token chunks
QC = 512
NKT = T // P     # 16 k tiles
f32 = mybir.dt.float32
bf16 = mybir.dt.bfloat16
Exp = mybir.ActivationFunctionType.Exp
ADD = mybir.AluOpType.add
MUL = mybir.AluOpType.mult

BF16 = ml_dtypes.bfloat16

_CACHE = {}


def _build(loop_reps=None):
    import contextlib
    nc = bacc.Bacc("TRN2", target_bir_lowering=False, debug=False)
    xT = nc.dram_tensor("xT", [C, T], bf16, kind="ExternalInput").ap()
    wqkT = nc.dram_tensor("wqkT", [C, 1024], bf16, kind="ExternalInput").ap()
    wvT = nc.dram_tensor("wvT", [C, 512], bf16, kind="ExternalInput").ap()
    wpT = nc.dram_tensor("wpT", [512, C], bf16, kind="ExternalInput").ap()
    bqk = nc.dram_tensor("bqk", [1024, 1], f32, kind="ExternalInput").ap()
    tri = nc.dram_tensor("tri", [P, P], f32, kind="ExternalInput").ap()
    out = nc.dram_tensor("out", [T, C], bf16, kind="ExternalOutput").ap()

    with tile.TileContext(nc) as tc:
        with tc.tile_pool(name="pers", bufs=1) as pers, \
             tc.tile_pool(name="xr", bufs=2) as xrp, \
             tc.tile_pool(name="qpool", bufs=2) as qpool, \
             tc.tile_pool(name="epool", bufs=3) as epool, \
             tc.tile_pool(name="fin", bufs=1) as fin, \
             tc.tile_pool(name="ypool", bufs=2) as ypool, \
             tc.tile_pool(name="opool", bufs=2) as opool, \
             tc.tile_pool(name="qk_ps", bufs=2, space="PSUM") as qk_ps, \
             tc.tile_pool(name="yA_ps", bufs=1, space="PSUM") as yA_ps, \
             tc.tile_pool(name="yB_ps", bufs=1, space="PSUM") as yB_ps, \
             tc.tile_pool(name="ac_ps", bufs=2, space="PSUM") as ac_ps, \
             (tc.For_i(0, loop_reps) if loop_reps else contextlib.nullcontext()):

            # ---- persistent weights: direct bf16 DMA, no staging ----
            wqk_sb = [pers.tile([P, 1024], bf16, tag=f"wqk{s}", name=f"wqk{s}")
                      for s in range(8)]
            wv_sb = [pers.tile([P, 512], bf16, tag=f"wv{s}", name=f"wv{s}")
                     for s in range(8)]
            wp_sb = [pers.tile([P, 1024], bf16, tag=f"wp{s}", name=f"wp{s}")
                     for s in range(4)]

            def load_weights_qkv():
                for s in range(8):
                    nc.sync.dma_start(wqk_sb[s][:], wqkT[s * P:(s + 1) * P, :])
                    yield
                for s in range(8):
                    nc.sync.dma_start(wv_sb[s][:], wvT[s * P:(s + 1) * P, :])
                    yield

            def load_weights_proj():
                for s in range(4):
                    nc.sync.dma_start(wp_sb[s][:], wpT[s * P:(s + 1) * P, :])
                    yield

            bqk_sb = pers.tile([P, 8], f32)
            tri_sb = pers.tile([P, P], f32)
            ones_sb = pers.tile([P, 8], bf16)
            warm_sb = pers.tile([1, 1], f32)

            def load_consts():
                # emitted after the first x DMAs so the exp-table preload and
                # const loads don't block the ACT/Pool DMA queues at t=0
                nc.gpsimd.dma_start(bqk_sb[:], bqk.rearrange("(m p) o -> p (m o)", p=P))
                nc.gpsimd.dma_start(tri_sb[:], tri)
                nc.vector.memset(ones_sb[:], 1.0)
                nc.scalar.activation(warm_sb[:], tri_sb[0:1, 0:1], Exp)

            # persistent activations
            k_sb = [pers.tile([P, T], bf16, tag=f"k{p}", name=f"k{p}") for p in range(4)]
            v_sb = [pers.tile([P, 8, 65], bf16, tag=f"v{t}", name=f"v{t}") for t in range(NKT)]
            q_tiles = {}   # (p, n) -> tile
            y_tiles = {}   # (p, n) -> tile

            xT_r = xT.rearrange("(s p) t -> p s t", p=P)

            def phase_a(n):
                xr = xrp.tile([P, 8, QC], bf16, tag="xr")
                if n == 0:
                    # slice-granular loads on two idle queues so the first
                    # slices land early (weights stream on SP in parallel)
                    for s in range(8):
                        eng = nc.scalar if s % 2 == 0 else nc.gpsimd
                        eng.dma_start(xr[:, s, :], xT_r[:, s, n * QC:(n + 1) * QC])
                else:
                    for hf in range(2):
                        nc.gpsimd.dma_start(xr[:, 4 * hf:4 * (hf + 1), :],
                                            xT_r[:, 4 * hf:4 * (hf + 1), n * QC:(n + 1) * QC])
                yield

                def emit_qk(m):
                    ps = ac_ps.tile([P, QC], f32, tag="acg")
                    for s in range(8):
                        nc.tensor.matmul(ps[:], wqk_sb[s][:, m * P:(m + 1) * P],
                                         xr[:, s, :], start=(s == 0), stop=(s == 7))
                        if s == 3:
                            yield
                    if m < 4:
                        qt = qpool.tile([P, QC], bf16, tag=f"q{m}")
                        nc.vector.tensor_scalar_add(qt[:], ps[:], bqk_sb[:, m:m + 1])
                        q_tiles[(m, n)] = qt
                    else:
                        nc.vector.tensor_scalar_add(k_sb[m - 4][:, n * QC:(n + 1) * QC],
                                                    ps[:], bqk_sb[:, m:m + 1])
                    yield

                def emit_v(ti):
                    t = 4 * n + ti
                    ps = ac_ps.tile([P, 8, D], f32, tag="acg")
                    for s in range(8):
                        nc.tensor.matmul(ps[:], xr[:, s, ti * P:(ti + 1) * P],
                                         wv_sb[s][:], start=(s == 0), stop=(s == 7))
                        if s == 3:
                            yield
                    nc.vector.tensor_copy(v_sb[t][:, :, 64:65], ones_sb[:, :, None])
                    nc.vector.tensor_copy(v_sb[t][:, :, 0:64], ps[:])
                    yield

                # pair-0 q/k first, then v tiles, then remaining pairs:
                # B(n) pair p unblocks as early as possible.
                yield from emit_qk(0)
                yield from emit_qk(4)
                for ti in range(4):
                    yield from emit_v(ti)
                for p in range(1, 4):
                    yield from emit_qk(p)
                    yield from emit_qk(4 + p)

            def phase_b(n):
                for p in range(4):
                    psy = [
                        yA_ps.tile([65, QC], f32, tag="psyA", name=f"psyA_{n}_{p}"),
                        yB_ps.tile([65, QC], f32, tag="psyB", name=f"psyB_{n}_{p}"),
                    ]
                    last = 4 * n + 3
                    qt = q_tiles[(p, n)]
                    for j in range(4 * n + 4):
                        diag = j >= 4 * n
                        o = P * (j - 4 * n) if diag else 0
                        ps_g = qk_ps.tile([P, 2, QC], f32, tag="qkg")
                        for h in range(2):
                            b0 = h * 64
                            nc.tensor.matmul(ps_g[:, h, o:], k_sb[p][b0:b0 + 64, j * P:(j + 1) * P],
                                             qt[b0:b0 + 64, o:], start=True, stop=True)
                        if diag:
                            nc.vector.tensor_tensor(
                                ps_g[:, :, o:o + P], ps_g[:, :, o:o + P],
                                tri_sb[:, None, :].to_broadcast((P, 2, P)), ADD)
                        e = epool.tile([P, 2, QC], bf16, tag="e")
                        nc.scalar.activation(e[:, :, o:], ps_g[:, :, o:], Exp)
                        for h in range(2):
                            nc.tensor.matmul(psy[h][:, o:], v_sb[j][:, 2 * p + h, :],
                                             e[:, h, o:], start=(j == 0), stop=(j == last))
                        yield
                    # per-head reciprocal -> partition_broadcast -> multiply.
                    # Each broadcast writes a full base-0 tile (the only
                    # broadcast shape that is correct on HW); the per-head
                    # split overlaps head-1's chain behind head-0's.
                    yt = ypool.tile([P, QC], bf16, tag=f"y{p}")
                    if n == 3 and p == 3:
                        # last pair gates C(3): finalize per 128-q subtile so
                        # C(3)'s first groups unblock after subtile 0
                        for ti in range(4):
                            sl = slice(ti * P, (ti + 1) * P)
                            for h in range(2):
                                rc = fin.tile([1, P], f32, tag=f"rs{h}")
                                nc.vector.reciprocal(rc[:], psy[h][64:65, sl])
                                rb = fin.tile([64, P], f32, tag=f"rbs{h}")
                                nc.gpsimd.partition_broadcast(rb[:], rc[:])
                                nc.vector.tensor_tensor(yt[h * 64:(h + 1) * 64, sl],
                                                        psy[h][0:64, sl], rb[:], MUL)
                    else:
                        for h in range(2):
                            rc = fin.tile([1, QC], f32, tag=f"r{h}")
                            nc.vector.reciprocal(rc[:], psy[h][64:65, :])
                            rb = fin.tile([64, QC], f32, tag=f"rb{h}")
                            nc.gpsimd.partition_broadcast(rb[:], rc[:])
                            nc.vector.tensor_tensor(yt[h * 64:(h + 1) * 64, :],
                                                    psy[h][0:64, :], rb[:], MUL)
                    y_tiles[(p, n)] = yt
                    yield

            def phase_c(n):
                for ti in range(4):
                    t = 4 * n + ti
                    for cc in range(2):
                        ps = ac_ps.tile([P, QC], f32, tag="acg")
                        for s in range(4):
                            nc.tensor.matmul(ps[:], y_tiles[(s, n)][:, ti * P:(ti + 1) * P],
                                             wp_sb[s][:, cc * QC:(cc + 1) * QC],
                                             start=(s == 0), stop=(s == 3))
                        ob = opool.tile([P, QC], bf16, tag="ob")
                        if n == 3:
                            # alternate the PSUM->SBUF copy between DVE and
                            # ACT so the tail is not DVE-serial
                            if (2 * ti + cc) % 2 == 0:
                                nc.vector.tensor_copy(ob[:], ps[:])
                                dma = nc.sync
                            else:
                                nc.scalar.activation(
                                    ob[:], ps[:], mybir.ActivationFunctionType.Copy)
                                dma = nc.scalar
                        else:
                            nc.vector.tensor_copy(ob[:], ps[:])
                            dma = (nc.sync, nc.gpsimd)[(2 * ti + cc) % 2]
                        dma.dma_start(out[t * P:(t + 1) * P, cc * QC:(cc + 1) * QC], ob[:])
                        yield

            def chain(*gens):
                for g in gens:
                    yield from g

            def run_all(g):
                for _ in g:
                    pass

            _SENTINEL = object()

            def interleave_lazy(base, inject, rate):
                """Emit all of `base`; after each base step emit `rate` steps
                of `inject` (fractional rates accumulate). Leftover inject
                steps are emitted at the end."""
                inj_iter = iter(inject)
                acc = 0.0
                exhausted = False
                for _ in base:
                    if exhausted:
                        continue
                    acc += rate
                    while acc >= 1.0 and not exhausted:
                        acc -= 1.0
                        if next(inj_iter, _SENTINEL) is _SENTINEL:
                            exhausted = True
                for _ in inj_iter:
                    pass

            # ---- emission schedule ----
            # prologue: x(0) DMAs first, then all qkv weight DMAs, then A(0)
            # through pair-0 q/k and the 4 v tiles.
            a0 = phase_a(0)
            next(a0)                           # all 8 x DMAs issued
            load_consts()
            run_all(load_weights_qkv())
            for _ in range(6):                 # qk(0), qk(4), v(0)
                next(a0)
            # B(n) yields: 4 * (4n+4 j-steps + 1 finalize)
            b_steps = [4 * (4 * n + 5) for n in range(4)]
            interleave_lazy(phase_b(0), chain(a0, load_weights_proj(), phase_a(1)),
                            (18 + 4 + 25) / b_steps[0])
            interleave_lazy(phase_b(1), chain(phase_a(2), phase_c(0)), 34 / b_steps[1])
            interleave_lazy(phase_b(2), chain(phase_a(3), phase_c(1)), 34 / b_steps[2])
            interleave_lazy(phase_b(3), phase_c(2), 8 / b_steps[3])
            run_all(phase_c(3))

    nc.compile()
    return nc


def _prep_core_inputs(c, x, w_attn, b_attn):
    b, g = divmod(c, 2)
    heads = [g * 8 + 2 * p + e for p in range(4) for e in range(2)]
    qrows = np.concatenate([np.arange(h * D, (h + 1) * D) for h in heads])
    # wqkT columns: q pairs (scaled 1/8) then k pairs
    wq = w_attn[qrows, :] * 0.125
    wk = w_attn[C + qrows, :]
    wqkT = np.ascontiguousarray(np.concatenate([wq, wk], 0).T)
    wvT = np.ascontiguousarray(w_attn[2 * C + qrows, :].T)
    bqk = np.concatenate([b_attn[qrows] * 0.125, b_attn[C + qrows]]).reshape(1024, 1)
    xTc = np.ascontiguousarray(x[b].T)
    return {
        "xT": xTc.astype(BF16),
        "wqkT": wqkT.astype(BF16),
        "wvT": wvT.astype(BF16),
        "bqk": bqk.astype(np.float32),
    }


def _prep_proj(c, w_proj):
    g = c % 2
    heads = [g * 8 + 2 * p + e for p in range(4) for e in range(2)]
    ch = np.concatenate([np.arange(h * D, (h + 1) * D) for h in heads])
    return np.ascontiguousarray(w_proj[:, ch].T).astype(BF16)


def _tri_mask():
    k = np.arange(P)[:, None]
    q = np.arange(P)[None, :]
    return np.where(q >= k, 0.0, -1e30).astype(np.float32)


def kernel(x, w_attn, b_attn, w_proj, b_proj):
    x = np.asarray(x, dtype=np.float32)
    w_attn = np.asarray(w_attn, dtype=np.float32)
    b_attn = np.asarray(b_attn, dtype=np.float32)
    w_proj = np.asarray(w_proj, dtype=np.float32)
    b_proj = np.asarray(b_proj, dtype=np.float32)

    if "nc" not in _CACHE:
        _CACHE["nc"] = _build()
    nc = _CACHE["nc"]

    tri = _tri_mask()
    in_maps = []
    for c in range(NCORES):
        m = _prep_core_inputs(c, x, w_attn, b_attn)
        m["wpT"] = _prep_proj(c, w_proj)
        m["tri"] = tri
        in_maps.append(m)

    res = bass_utils.run_bass_kernel_spmd(nc, in_maps, core_ids=list(range(NCORES)))
    outs = [r["out"] for r in res.results]

    B = x.shape[0]
    corr = (b_attn[2 * C:] @ w_proj.T + b_proj).astype(np.float32)
    full = np.empty((B, T, C), np.float32)
    for b in range(B):
        full[b] = (outs[2 * b].astype(np.float32)
                   + outs[2 * b + 1].astype(np.float32) + corr)
    return full


# revision 27
# speedup vs baseline: 1.2053x; 1.2053x over previous
"""Causal self-attention (B=4, T=2048, C=1024, H=16, D=64) on 8 TRN2 NeuronCores.

Sharding: core c = 2*b + g handles batch b (0..3) and head-group g (0..1),
i.e. 8 heads per core (4 head-pairs). Column-parallel QKV, row-parallel
c_proj; host sums the two partial outputs per batch.

All matmul operands are bf16 (host pre-casts x and weights; DMA loads them
straight into SBUF with no staging copies), accumulation is fp32 in PSUM.
A/C projection phases share one 2-buffer PSUM pool so consecutive
accumulation groups ping-pong banks instead of serializing.

Per-core pipeline (chunk-pipelined A->B->C, interleaved at sub-tile
granularity so the PE/ACT streams stay dense):
  A(n): QKV projection for 512-token chunk n. qT/kT as [pair dims, tokens],
        v as [tokens, head, dim(+ones col)].
  B(n): flash attention, scores transposed attT[k, q]; exp on ACT
        (PSUM->SBUF, bf16 out), AV with ones-augmented V accumulating yT +
        denominator; deferred normalization via DVE reciprocal +
        GpSimd partition_broadcast + DVE multiply.
  C(n): row-parallel c_proj partial output for chunk n.

Biases: b_attn q/k parts added on-device (per-partition scalar add);
v-bias and b_proj folded into a host-side output correction.
"""

import numpy as np
import ml_dtypes

import concourse.bass as bass
from concourse import bacc, tile, mybir, bass_utils

P = 128
T = 2048
C = 1024
NH = 16          # total heads
D = 64
NCORES = 8
NCH = 4          # 512-token chunks
QC = 512
NKT = T // P     # 16 k tiles
f32 = mybir.dt.float32
bf16 = mybir.dt.bfloat16
Exp = mybir.ActivationFunctionType.Exp
ADD = mybir.AluOpType.add
MUL = mybir.AluOpType.mult

BF16 = ml_dtypes.bfloat16

_CACHE = {}


def _build(loop_reps=None):
    import contextlib
    nc = bacc.Bacc("TRN2", target_bir_lowering=False, debug=False)
    xT = nc.dram_tensor("xT", [C, T], bf16, kind="ExternalInput").ap()
    wqkT = nc.dram_tensor("wqkT", [C, 1024], bf16, kind="ExternalInput").ap()
    wvT = nc.dram_tensor("wvT", [C, 512], bf16, kind="ExternalInput").ap()
    wpT = nc.dram_tensor("wpT", [512, C], bf16, kind="ExternalInput").ap()
    bqk = nc.dram_tensor("bqk", [1024, 1], f32, kind="ExternalInput").ap()
    tri = nc.dram_tensor("tri", [P, P], f32, kind="ExternalInput").ap()
    out = nc.dram_tensor("out", [T, C], bf16, kind="ExternalOutput").ap()

    with tile.TileContext(nc) as tc:
        with tc.tile_pool(name="pers", bufs=1) as pers, \
             tc.tile_pool(name="xr", bufs=2) as xrp, \
             tc.tile_pool(name="qpool", bufs=2) as qpool, \
             tc.tile_pool(name="epool", bufs=3) as epool, \
             tc.tile_pool(name="fin", bufs=1) as fin, \
             tc.tile_pool(name="ypool", bufs=3) as ypool, \
             tc.tile_pool(name="opool", bufs=2) as opool, \
             tc.tile_pool(name="qk_ps", bufs=2, space="PSUM") as qk_ps, \
             tc.tile_pool(name="yA_ps", bufs=1, space="PSUM") as yA_ps, \
             tc.tile_pool(name="yB_ps", bufs=1, space="PSUM") as yB_ps, \
             tc.tile_pool(name="ac_ps", bufs=2, space="PSUM") as ac_ps, \
             (tc.For_i(0, loop_reps) if loop_reps else contextlib.nullcontext()):

            # ---- persistent weights: direct bf16 DMA, no staging ----
            wqk_sb = [pers.tile([P, 1024], bf16, tag=f"wqk{s}", name=f"wqk{s}")
                      for s in range(8)]
            wv_sb = [pers.tile([P, 512], bf16, tag=f"wv{s}", name=f"wv{s}")
                     for s in range(8)]
            wp_sb = [pers.tile([P, 1024], bf16, tag=f"wp{s}", name=f"wp{s}")
                     for s in range(4)]

            def load_weights_qkv():
                for s in range(8):
                    nc.sync.dma_start(wqk_sb[s][:], wqkT[s * P:(s + 1) * P, :])
                    yield
                for s in range(8):
                    nc.sync.dma_start(wv_sb[s][:], wvT[s * P:(s + 1) * P, :])
                    yield

            def load_weights_proj():
                for s in range(4):
                    nc.sync.dma_start(wp_sb[s][:], wpT[s * P:(s + 1) * P, :])
                    yield

            bqk_sb = pers.tile([P, 8], f32)
            tri_sb = pers.tile([P, P], f32)
            ones_sb = pers.tile([P, 8], bf16)
            warm_sb = pers.tile([1, 1], f32)

            def load_consts():
                # emitted after the first x DMAs so the exp-table preload and
                # const loads don't block the ACT/Pool DMA queues at t=0
                nc.gpsimd.dma_start(bqk_sb[:], bqk.rearrange("(m p) o -> p (m o)", p=P))
                nc.gpsimd.dma_start(tri_sb[:], tri)
                nc.vector.memset(ones_sb[:], 1.0)
                nc.scalar.activation(warm_sb[:], tri_sb[0:1, 0:1], Exp)

            # persistent activations
            k_sb = [pers.tile([P, T], bf16, tag=f"k{p}", name=f"k{p}") for p in range(4)]
            v_sb = [pers.tile([P, 8, 65], bf16, tag=f"v{t}", name=f"v{t}") for t in range(NKT)]
            q_tiles = {}   # (p, n) -> tile
            y_tiles = {}   # (p, n) -> tile

            xT_r = xT.rearrange("(s p) t -> p s t", p=P)

            def phase_a(n):
                xr = xrp.tile([P, 8, QC], bf16, tag="xr")
                if n == 0:
                    # slice-granular loads on two idle queues so the first
                    # slices land early (weights stream on SP in parallel)
                    for s in range(8):
                        eng = nc.scalar if s % 2 == 0 else nc.gpsimd
                        eng.dma_start(xr[:, s, :], xT_r[:, s, n * QC:(n + 1) * QC])
                else:
                    for hf in range(2):
                        nc.gpsimd.dma_start(xr[:, 4 * hf:4 * (hf + 1), :],
                                            xT_r[:, 4 * hf:4 * (hf + 1), n * QC:(n + 1) * QC])
                yield

                def emit_qk(m):
                    ps = ac_ps.tile([P, QC], f32, tag="acg")
                    for s in range(8):
                        nc.tensor.matmul(ps[:], wqk_sb[s][:, m * P:(m + 1) * P],
                                         xr[:, s, :], start=(s == 0), stop=(s == 7))
                        if s == 3:
                            yield
                    if m < 4:
                        qt = qpool.tile([P, QC], bf16, tag=f"q{m}")
                        nc.vector.tensor_scalar_add(qt[:], ps[:], bqk_sb[:, m:m + 1])
                        q_tiles[(m, n)] = qt
                    else:
                        nc.vector.tensor_scalar_add(k_sb[m - 4][:, n * QC:(n + 1) * QC],
                                                    ps[:], bqk_sb[:, m:m + 1])
                    yield

                def emit_v(ti):
                    t = 4 * n + ti
                    ps = ac_ps.tile([P, 8, D], f32, tag="acg")
                    for s in range(8):
                        nc.tensor.matmul(ps[:], xr[:, s, ti * P:(ti + 1) * P],
                                         wv_sb[s][:], start=(s == 0), stop=(s == 7))
                        if s == 3:
                            yield
                    nc.vector.tensor_copy(v_sb[t][:, :, 64:65], ones_sb[:, :, None])
                    nc.vector.tensor_copy(v_sb[t][:, :, 0:64], ps[:])
                    yield

                # pair-0 q/k first, then v tiles, then remaining pairs:
                # B(n) pair p unblocks as early as possible.
                yield from emit_qk(0)
                yield from emit_qk(4)
                for ti in range(4):
                    yield from emit_v(ti)
                for p in range(1, 4):
                    yield from emit_qk(p)
                    yield from emit_qk(4 + p)

            def phase_b(n):
                for p in range(4):
                    psy = [
                        yA_ps.tile([65, QC], f32, tag="psyA", name=f"psyA_{n}_{p}"),
                        yB_ps.tile([65, QC], f32, tag="psyB", name=f"psyB_{n}_{p}"),
                    ]
                    last = 4 * n + 3
                    qt = q_tiles[(p, n)]
                    for j in range(4 * n + 4):
                        diag = j >= 4 * n
                        o = P * (j - 4 * n) if diag else 0
                        ps_g = qk_ps.tile([P, 2, QC], f32, tag="qkg")
                        for h in range(2):
                            b0 = h * 64
                            nc.tensor.matmul(ps_g[:, h, o:], k_sb[p][b0:b0 + 64, j * P:(j + 1) * P],
                                             qt[b0:b0 + 64, o:], start=True, stop=True)
                        if diag:
                            nc.vector.tensor_tensor(
                                ps_g[:, :, o:o + P], ps_g[:, :, o:o + P],
                                tri_sb[:, None, :].to_broadcast((P, 2, P)), ADD)
                        e = epool.tile([P, 2, QC], bf16, tag="e")
                        nc.scalar.activation(e[:, :, o:], ps_g[:, :, o:], Exp)
                        for h in range(2):
                            nc.tensor.matmul(psy[h][:, o:], v_sb[j][:, 2 * p + h, :],
                                             e[:, h, o:], start=(j == 0), stop=(j == last))
                        yield
                    # per-head reciprocal -> partition_broadcast -> multiply.
                    # Each broadcast writes a full base-0 tile (the only
                    # broadcast shape that is correct on HW); the per-head
                    # split overlaps head-1's chain behind head-0's.
                    yt = ypool.tile([P, QC], bf16, tag=f"y{p}")
                    if n == 3 and p == 3:
                        # last pair gates C(3): finalize per 128-q subtile so
                        # C(3)'s first groups unblock after subtile 0
                        for ti in range(4):
                            sl = slice(ti * P, (ti + 1) * P)
                            for h in range(2):
                                rc = fin.tile([1, P], f32, tag=f"rs{h}")
                                nc.vector.reciprocal(rc[:], psy[h][64:65, sl])
                                rb = fin.tile([64, P], f32, tag=f"rbs{h}")
                                nc.gpsimd.partition_broadcast(rb[:], rc[:])
                                nc.vector.tensor_tensor(yt[h * 64:(h + 1) * 64, sl],
                                                        psy[h][0:64, sl], rb[:], MUL)
                    else:
                        for h in range(2):
                            rc = fin.tile([1, QC], f32, tag=f"r{h}")
                            nc.vector.reciprocal(rc[:], psy[h][64:65, :])
                            rb = fin.tile([64, QC], f32, tag=f"rb{h}")
                            nc.gpsimd.partition_broadcast(rb[:], rc[:])
                            nc.vector.tensor_tensor(yt[h * 64:(h + 1) * 64, :],
                                                    psy[h][0:64, :], rb[:], MUL)
                    y_tiles[(p, n)] = yt
                    yield

            def phase_c(n):
                for ti in range(4):
                    t = 4 * n + ti
                    for cc in range(2):
                        ps = ac_ps.tile([P, QC], f32, tag="acg")
                        for s in range(4):
                            nc.tensor.matmul(ps[:], y_tiles[(s, n)][:, ti * P:(ti + 1) * P],
                                             wp_sb[s][:, cc * QC:(cc + 1) * QC],
                                             start=(s == 0), stop=(s == 3))
                        ob = opool.tile([P, QC], bf16, tag="ob")
                        if n == 3:
                            # alternate the PSUM->SBUF copy between DVE and
                            # ACT so the tail is not DVE-serial
                            if (2 * ti + cc) % 2 == 0:
                                nc.vector.tensor_copy(ob[:], ps[:])
                                dma = nc.sync
                            else:
                                nc.scalar.activation(
                                    ob[:], ps[:], mybir.ActivationFunctionType.Copy)
                                dma = nc.scalar
                        else:
                            nc.vector.tensor_copy(ob[:], ps[:])
                            dma = (nc.sync, nc.gpsimd)[(2 * ti + cc) % 2]
                        dma.dma_start(out[t * P:(t + 1) * P, cc * QC:(cc + 1) * QC], ob[:])
                        yield

            def chain(*gens):
                for g in gens:
                    yield from g

            def run_all(g):
                for _ in g:
                    pass

            _SENTINEL = object()

            def interleave_lazy(base, inject, rate):
                """Emit all of `base`; after each base step emit `rate` steps
                of `inject` (fractional rates accumulate). Leftover inject
                steps are emitted at the end."""
                inj_iter = iter(inject)
                acc = 0.0
                exhausted = False
                for _ in base:
                    if exhausted:
                        continue
                    acc += rate
                    while acc >= 1.0 and not exhausted:
                        acc -= 1.0
                        if next(inj_iter, _SENTINEL) is _SENTINEL:
                            exhausted = True
                for _ in inj_iter:
                    pass

            # ---- emission schedule ----
            # prologue: x(0) DMAs first, then all qkv weight DMAs, then A(0)
            # through pair-0 q/k and the 4 v tiles.
            a0 = phase_a(0)
            next(a0)                           # all 8 x DMAs issued
            load_consts()
            run_all(load_weights_qkv())
            for _ in range(6):                 # qk(0), qk(4), v(0)
                next(a0)
            # B(n) yields: 4 * (4n+4 j-steps + 1 finalize)
            b_steps = [4 * (4 * n + 5) for n in range(4)]
            interleave_lazy(phase_b(0), chain(a0, load_weights_proj(), phase_a(1)),
                            (18 + 4 + 25) / b_steps[0])
            interleave_lazy(phase_b(1), chain(phase_a(2), phase_c(0)), 34 / b_steps[1])
            interleave_lazy(phase_b(2), phase_a(3), 25 / b_steps[2])
            interleave_lazy(phase_b(3), chain(phase_c(1), phase_c(2)), 16 / b_steps[3])
            run_all(phase_c(3))

    nc.compile()
    return nc


def _prep_core_inputs(c, x, w_attn, b_attn):
    b, g = divmod(c, 2)
    heads = [g * 8 + 2 * p + e for p in range(4) for e in range(2)]
    qrows = np.concatenate([np.arange(h * D, (h + 1) * D) for h in heads])
    # wqkT columns: q pairs (scaled 1/8) then k pairs
    wq = w_attn[qrows, :] * 0.125
    wk = w_attn[C + qrows, :]
    wqkT = np.ascontiguousarray(np.concatenate([wq, wk], 0).T)
    wvT = np.ascontiguousarray(w_attn[2 * C + qrows, :].T)
    bqk = np.concatenate([b_attn[qrows] * 0.125, b_attn[C + qrows]]).reshape(1024, 1)
    xTc = np.ascontiguousarray(x[b].T)
    return {
        "xT": xTc.astype(BF16),
        "wqkT": wqkT.astype(BF16),
        "wvT": wvT.astype(BF16),
        "bqk": bqk.astype(np.float32),
    }


def _prep_proj(c, w_proj):
    g = c % 2
    heads = [g * 8 + 2 * p + e for p in range(4) for e in range(2)]
    ch = np.concatenate([np.arange(h * D, (h + 1) * D) for h in heads])
    return np.ascontiguousarray(w_proj[:, ch].T).astype(BF16)


def _tri_mask():
    k = np.arange(P)[:, None]
    q = np.arange(P)[None, :]
    return np.where(q >= k, 0.0, -1e30).astype(np.float32)


def kernel(x, w_attn, b_attn, w_proj, b_proj):
    x = np.asarray(x, dtype=np.float32)
    w_attn = np.asarray(w_attn, dtype=np.float32)
    b_attn = np.asarray(b_attn, dtype=np.float32)
    w_proj = np.asarray(w_proj, dtype=np.float32)
    b_proj = np.asarray(b_proj, dtype=np.float32)

    if "nc" not in _CACHE:
        _CACHE["nc"] = _build()
    nc = _CACHE["nc"]

    tri = _tri_mask()
    in_maps = []
    for c in range(NCORES):
        m = _prep_core_inputs(c, x, w_attn, b_attn)
        m["wpT"] = _prep_proj(c, w_proj)
        m["tri"] = tri
        in_maps.append(m)

    res = bass_utils.run_bass_kernel_spmd(nc, in_maps, core_ids=list(range(NCORES)))
    outs = [r["out"] for r in res.results]

    B = x.shape[0]
    corr = (b_attn[2 * C:] @ w_proj.T + b_proj).astype(np.float32)
    full = np.empty((B, T, C), np.float32)
    for b in range(B):
        full[b] = (outs[2 * b].astype(np.float32)
                   + outs[2 * b + 1].astype(np.float32) + corr)
    return full
